# revision 1
# baseline (speedup 1.0000x reference)
"""K-center farthest-point step on 8 Trainium2 NeuronCores.

Computes, for x[16384,512], y[16384,512]:
    dists = cdist(x, y); min_d = dists.min(axis=1)
    return (min_d.max(), min_d.argmax())

Strategy (per sharding hint): shard x rows across 8 cores (2048 rows each),
replicate y. The host passes y pre-transposed (d-major) plus precomputed
||y_j||^2, so each core streams y^T tiles straight into fp32r matmuls
(full-rate PE) fused with a per-partition add + running-min on the vector
engine: m[i] = min_j(||y_j||^2 - 2 x_i . y_j). The host adds ||x_i||^2,
gathers the 8 shards, and resolves the argmax with an exact-fp32 top-K
refinement so fp32r rounding cannot flip the result.
"""

import sys

sys.path.insert(0, "/opt/trn_rl_repo")

import numpy as np

N, D = 16384, 512
NCORES = 8
SHARD = N // NCORES  # 2048
NI = SHARD // 512    # 4 moving i-chunks per core
ND = D // 128        # 4 contraction chunks
NJ = N // 128        # 128 j tiles

_CACHE = {}


def _build_bass():
    import concourse.bass as bass
    import concourse.mybir as mybir
    import concourse.tile as tile
    from concourse.masks import make_identity

    f32 = mybir.dt.float32
    f32r = mybir.dt.float32r
    Alu = mybir.AluOpType

    nc = bass.Bass(trn_type="TRN2")
    x_d = nc.dram_tensor("x", [SHARD, D], f32, kind="ExternalInput")
    yT_d = nc.dram_tensor("yT", [D, N], f32, kind="ExternalInput")
    ysq_d = nc.dram_tensor("ysqT", [128, NJ], f32, kind="ExternalInput")
    out_d = nc.dram_tensor("out", [128, SHARD], f32, kind="ExternalOutput")

    with tile.TileContext(nc) as tc:
        with (
            tc.tile_pool(name="persist", bufs=1) as persist,
            tc.tile_pool(name="xnat", bufs=8) as xnat_p,
            tc.tile_pool(name="yT", bufs=8) as yT_p,
            tc.tile_pool(name="pg", bufs=8, space="PSUM") as pg_p,
        ):
            ident_f = persist.tile([128, 128], f32)
            make_identity(nc, ident_f[:])
            ident = persist.tile([128, 128], f32r)
            nc.scalar.copy(ident[:], ident_f[:])

            # persistent: xT[d] = -2 x^T chunk (f32r), [128 d, SHARD i]
            xT = [
                persist.tile([128, SHARD], f32r, name=f"xT{d}", tag=f"xT{d}")
                for d in range(ND)
            ]
            macc = persist.tile([128, SHARD], f32)
            nc.vector.memset(macc[:], 3.0e38)
            ysq_all = persist.tile([128, NJ], f32)
            nc.sync.dma_start(out=ysq_all[:], in_=ysq_d[:])

            # ---- pre-issue first y^T tile DMAs so they aren't queued
            # behind the whole 4MB x preamble on the DMA FIFO ----
            yTj_pre = {}
            for jt in range(4):
                ytile = yT_p.tile(
                    [128, 512], f32r, name=f"yTpre{jt}", tag="yTj"
                )
                nc.sync.dma_start(
                    out=ytile[:].rearrange("p (d j) -> p d j", d=ND),
                    in_=yT_d.rearrange("(d p) n -> p d n", p=128)[
                        :, :, jt * 128:(jt + 1) * 128
                    ].bitcast(f32r),
                )
                yTj_pre[jt] = ytile

            # ---- preamble: load x shard, transpose, scale by -2 ----
            for it in range(SHARD // 128):  # 16
                xnat = xnat_p.tile([128, D], f32r)
                nc.sync.dma_start(
                    out=xnat[:],
                    in_=x_d[it * 128:(it + 1) * 128, :].bitcast(f32r),
                )
                pt = pg_p.tile([128, 512], f32r, name=f"ptx{it}", tag="pg")
                for d in range(ND):
                    nc.tensor.transpose(
                        pt[:, d * 128:(d + 1) * 128],
                        xnat[:, d * 128:(d + 1) * 128],
                        ident[:],
                    )
                for d in range(ND):
                    nc.vector.tensor_scalar_mul(
                        xT[d][:, it * 128:(it + 1) * 128],
                        pt[:, d * 128:(d + 1) * 128],
                        -2.0,
                    )

            # ---- main loop over y^T tiles (no on-chip transposes) ----
            for jt in range(NJ):  # 128
                # yTj[p, d*128 + j] = yT[d*128 + p, jt*128 + j]
                if jt in yTj_pre:
                    yTj = yTj_pre.pop(jt)
                else:
                    yTj = yT_p.tile([128, 512], f32r, name=f"yTj{jt}", tag="yTj")
                    nc.sync.dma_start(
                        out=yTj[:].rearrange("p (d j) -> p d j", d=ND),
                        in_=yT_d.rearrange("(d p) n -> p d n", p=128)[
                            :, :, jt * 128:(jt + 1) * 128
                        ].bitcast(f32r),
                    )

                pgs = [
                    pg_p.tile([128, 512], f32, name=f"pg{jt}_{s}", tag="pg")
                    for s in range(NI)
                ]
                for d in range(ND):  # 4 — stationary yTj[d] reused 4x
                    for s in range(NI):  # 4 moving 512-slices
                        nc.tensor.matmul(
                            pgs[s][:],
                            yTj[:, d * 128:(d + 1) * 128],
                            xT[d][:, s * 512:(s + 1) * 512],
                            start=(d == 0),
                            stop=(d == ND - 1),
                        )
                for s in range(NI):
                    # macc = min(macc, pg + ysq)  (ysq per-partition)
                    nc.vector.scalar_tensor_tensor(
                        out=macc[:, s * 512:(s + 1) * 512],
                        in0=pgs[s][:],
                        scalar=ysq_all[:, jt:jt + 1],
                        in1=macc[:, s * 512:(s + 1) * 512],
                        op0=Alu.add,
                        op1=Alu.min,
                    )

            for s in range(NI):
                nc.sync.dma_start(
                    out=out_d[:, s * 512:(s + 1) * 512],
                    in_=macc[:, s * 512:(s + 1) * 512],
                )

    return nc


def _split_multiwait_bir(raw: bytes) -> bytes:
    """Walrus codegen in this image rejects instructions with >1 sem wait
    ("Too many sync wait commands"). Split each multi-wait instruction into
    a chain of single-wait EventSemaphore instructions (same engine,
    in-order execution makes this equivalent) followed by the original
    instruction with at most one wait."""
    import orjson

    bir = orjson.loads(raw)
    uid = [0]
    for fn in bir.get("functions", []):
        for bb in fn.get("blocks", []):
            insts = bb.get("instructions", [])
            out = []
            for ins in insts:
                si = ins.get("sync_info") or {}
                waits = si.get("on_wait") or []
                if len(waits) > 1:
                    for w in waits[:-1]:
                        uid[0] += 1
                        out.append({
                            "debug": ins.get("debug", 0),
                            "engine": ins["engine"],
                            "ins": [],
                            "name": f"{ins['name']}__sw{uid[0]}",
                            "opcode": "EventSemaphore",
                            "outs": [],
                            "sync_info": {"on_update": [], "on_wait": [w]},
                        })
                    si["on_wait"] = [waits[-1]]
                out.append(ins)
            bb["instructions"] = out
    return orjson.dumps(bir)


def _get_nc():
    if "nc" not in _CACHE:
        nc = _build_bass()
        orig = nc.to_json_bytes
        nc.to_json_bytes = lambda: _split_multiwait_bir(orig())
        _CACHE["nc"] = nc
    return _CACHE["nc"]


def kernel(x, y, device=0, _want_profile=False):
    from concourse.bass_utils import run_bass_kernel_spmd

    x = np.ascontiguousarray(np.asarray(x, dtype=np.float32))
    y = np.ascontiguousarray(np.asarray(y, dtype=np.float32))
    assert x.shape == (N, D) and y.shape == (N, D)

    yT = np.ascontiguousarray(y.T)                      # [D, N]
    ysq = (y * y).sum(axis=1).astype(np.float32)        # [N]
    # ysqT[p, jt] = ysq[jt*128 + p]
    ysqT = np.ascontiguousarray(ysq.reshape(NJ, 128).T)

    nc = _get_nc()
    in_maps = [
        {"x": x[c * SHARD:(c + 1) * SHARD], "yT": yT, "ysqT": ysqT}
        for c in range(NCORES)
    ]
    try:
        res = run_bass_kernel_spmd(
            nc, in_maps, list(range(NCORES)), trace=_want_profile
        )
    except ModuleNotFoundError:
        res = run_bass_kernel_spmd(nc, in_maps, list(range(NCORES)))
    if _want_profile:
        _CACHE["exec_time_ns"] = getattr(res, "exec_time_ns", None)

    # per-core [128, SHARD] -> min over partitions -> [SHARD]
    parts = [res.results[c]["out"].min(axis=0) for c in range(NCORES)]
    m = np.concatenate(parts)  # [N] = min_j(||y_j||^2 - 2 x_i . y_j)

    xsq = (x * x).sum(axis=1)
    md2 = xsq + m  # squared min distances (fp32r-accurate)

    # exact fp32 top-K refinement: recompute candidate rows exactly so
    # fp32r rounding cannot flip the argmax.
    K = 128
    cand = np.argpartition(-md2, K)[:K]
    g = x[cand] @ y.T  # [K, N] exact fp32 (BLAS)
    d2 = xsq[cand][:, None] + ysq[None, :] - 2.0 * g
    cmin = d2.min(axis=1)
    best = int(np.argmax(cmin))
    max_id = int(cand[best])
    max_val = np.sqrt(np.maximum(cmin[best], 0.0), dtype=np.float32)

    return np.float32(max_val), np.int32(max_id)



# revision 2
# speedup vs baseline: 11.5215x; 11.5215x over previous
"""K-center farthest-point step on 8 Trainium2 NeuronCores.

Computes, for x[16384,512], y[16384,512]:
    dists = cdist(x, y); min_d = dists.min(axis=1)
    return (min_d.max(), min_d.argmax())

v2 strategy: the axon tunnel to the device runs at ~20-40 MB/s, so wire
bytes dominate end-to-end time. Ship each input element exactly once in
bf16: core c receives x rows [c*2048,(c+1)*2048) pre-scaled by -2 and y
rows [c*2048,(c+1)*2048) (4 MB/core, 32 MB total vs 288 MB for the fp32
replicate-y baseline). On device an AllGather over NeuronLink (~20 us)
assembles the full y; each core then computes its shard's row-mins
m[i] = min_j(||y_j||^2 - 2 x_i . y_j) with bf16 matmuls (fp32 PSUM) and
reduces over partitions on-chip, returning just [128,16] f32 per core.
The host adds ||x_i||^2, gathers the 8 shards, and resolves max/argmax
with an exact-fp32 top-K refinement so reduced-precision device math
cannot flip the result.
"""

import sys

sys.path.insert(0, "/opt/trn_rl_repo")

import numpy as np
import ml_dtypes

N, D = 16384, 512
NCORES = 8
SHARD = N // NCORES  # 2048
NI = SHARD // 512    # 4 moving i-chunks per core
ND = D // 128        # 4 contraction chunks
NJ = N // 128        # 128 j tiles
NT = SHARD // 128    # 16 i-tiles per core

_CACHE = {}


def _build_bass():
    import concourse.bass as bass
    import concourse.mybir as mybir
    import concourse.tile as tile
    from concourse.masks import make_identity

    f32 = mybir.dt.float32
    f32r = mybir.dt.float32r
    bf16 = mybir.dt.bfloat16
    Alu = mybir.AluOpType

    nc = bass.Bass(trn_type="TRN2", num_devices=NCORES)
    x_d = nc.dram_tensor("x", [SHARD, D], bf16, kind="ExternalInput")
    y_d = nc.dram_tensor("y", [SHARD, D], bf16, kind="ExternalInput")
    ysq_d = nc.dram_tensor("ysqT", [128, NJ], f32, kind="ExternalInput")
    out_d = nc.dram_tensor("out", [1, SHARD], f32, kind="ExternalOutput")

    with tile.TileContext(nc) as tc:
        with (
            tc.tile_pool(name="persist", bufs=1) as persist,
            tc.tile_pool(name="xnat", bufs=4) as xnat_p,
            tc.tile_pool(name="ytile", bufs=4) as ytile_p,
            tc.tile_pool(name="yTj", bufs=4) as yTj_p,
            tc.tile_pool(name="pg", bufs=4, space="PSUM") as pg_p,
            tc.tile_pool(name="tp", bufs=2, space="PSUM") as tp_p,
            tc.tile_pool(name="dram", bufs=1, space="DRAM") as dram_p,
        ):
            # ---- kick off the y AllGather as early as possible ----
            y_bounce = dram_p.tile([SHARD, D], bf16, name="y_bounce")
            y_full = dram_p.tile([N, D], bf16, name="y_full",
                                 addr_space="Shared")
            nc.gpsimd.dma_start(out=y_bounce[:], in_=y_d[:])
            nc.gpsimd.collective_compute(
                "AllGather",
                Alu.bypass,
                replica_groups=[list(range(NCORES))],
                ins=[y_bounce[:]],
                outs=[y_full[:]],
            )

            ident_f = persist.tile([128, 128], f32)
            make_identity(nc, ident_f[:])
            ident_b = persist.tile([128, 128], bf16)
            nc.scalar.copy(ident_b[:], ident_f[:])

            macc = persist.tile([128, SHARD], f32)
            nc.vector.memset(macc[:], 3.0e38)
            ysq_all = persist.tile([128, NJ], f32)
            nc.sync.dma_start(out=ysq_all[:], in_=ysq_d[:])

            # ---- preamble: load x shard (-2x in bf16), transpose on PE ----
            # xT[d][p, i] = -2 * x[i, d*128+p], bf16
            xT = [
                persist.tile([128, SHARD], bf16, name=f"xT{d}", tag=f"xT{d}")
                for d in range(ND)
            ]
            for it in range(NT):  # 16
                xnat = xnat_p.tile([128, D], bf16)
                nc.sync.dma_start(
                    out=xnat[:], in_=x_d[it * 128:(it + 1) * 128, :]
                )
                pt = tp_p.tile([128, D], bf16, name=f"ptx{it}", tag="tp")
                for d in range(ND):
                    nc.tensor.transpose(
                        pt[:, d * 128:(d + 1) * 128],
                        xnat[:, d * 128:(d + 1) * 128],
                        ident_b[:],
                    )
                for d in range(ND):
                    nc.scalar.copy(
                        xT[d][:, it * 128:(it + 1) * 128],
                        pt[:, d * 128:(d + 1) * 128],
                    )

            # ---- main loop over y tiles from the AllGathered y ----
            for jt in range(NJ):  # 128
                ytile = ytile_p.tile([128, D], bf16, name=f"yt{jt}", tag="yt")
                nc.sync.dma_start(
                    out=ytile[:], in_=y_full[jt * 128:(jt + 1) * 128, :]
                )
                ypt = tp_p.tile([128, D], bf16, name=f"ypt{jt}", tag="tp")
                for d in range(ND):
                    nc.tensor.transpose(
                        ypt[:, d * 128:(d + 1) * 128],
                        ytile[:, d * 128:(d + 1) * 128],
                        ident_b[:],
                    )
                # yTj[p, d*128+j] = y[jt*128+j, d*128+p], bf16
                yTj = yTj_p.tile([128, D], bf16, name=f"yTj{jt}", tag="yTj")
                nc.scalar.copy(yTj[:], ypt[:])

                pgs = [
                    pg_p.tile([128, 512], f32, name=f"pg{jt}_{s}", tag="pg")
                    for s in range(NI)
                ]
                for d in range(ND):  # stationary yTj chunk reused NI times
                    for s in range(NI):
                        nc.tensor.matmul(
                            pgs[s][:],
                            yTj[:, d * 128:(d + 1) * 128],
                            xT[d][:, s * 512:(s + 1) * 512],
                            start=(d == 0),
                            stop=(d == ND - 1),
                        )
                for s in range(NI):
                    # macc = min(macc, pg + ysq_j)  (ysq per-partition)
                    nc.vector.scalar_tensor_tensor(
                        out=macc[:, s * 512:(s + 1) * 512],
                        in0=pgs[s][:],
                        scalar=ysq_all[:, jt:jt + 1],
                        in1=macc[:, s * 512:(s + 1) * 512],
                        op0=Alu.add,
                        op1=Alu.min,
                    )

            # ---- epilogue: partition-min by log2 folding (exact f32);
            # DVE needs equal base partitions, so bounce the upper half
            # down to partition 0 with an SBUF->SBUF DMA each step ----
            cur = 128
            while cur > 1:
                h = cur // 2
                fold = xnat_p.tile([h, SHARD], f32, name=f"fold{h}", tag="fold")
                nc.sync.dma_start(out=fold[:], in_=macc[h:cur, :])
                nc.vector.tensor_tensor(
                    out=macc[:h, :],
                    in0=macc[:h, :],
                    in1=fold[:],
                    op=Alu.min,
                )
                cur = h
            nc.sync.dma_start(out=out_d[:], in_=macc[0:1, :])

    return nc


def _split_multiwait_bir(raw: bytes) -> bytes:
    """Walrus codegen in this image rejects instructions with >1 sem wait
    ("Too many sync wait commands"). Split each multi-wait instruction into
    a chain of single-wait EventSemaphore instructions (same engine,
    in-order execution makes this equivalent) followed by the original
    instruction with at most one wait."""
    import orjson

    bir = orjson.loads(raw)
    uid = [0]
    for fn in bir.get("functions", []):
        for bb in fn.get("blocks", []):
            insts = bb.get("instructions", [])
            out = []
            for ins in insts:
                si = ins.get("sync_info") or {}
                waits = si.get("on_wait") or []
                if len(waits) > 1:
                    for w in waits[:-1]:
                        uid[0] += 1
                        out.append({
                            "debug": ins.get("debug", 0),
                            "engine": ins["engine"],
                            "ins": [],
                            "name": f"{ins['name']}__sw{uid[0]}",
                            "opcode": "EventSemaphore",
                            "outs": [],
                            "sync_info": {"on_update": [], "on_wait": [w]},
                        })
                    si["on_wait"] = [waits[-1]]
                out.append(ins)
            bb["instructions"] = out
    return orjson.dumps(bir)


def _get_nc():
    if "nc" not in _CACHE:
        nc = _build_bass()
        orig = nc.to_json_bytes
        nc.to_json_bytes = lambda: _split_multiwait_bir(orig())
        _CACHE["nc"] = nc
    return _CACHE["nc"]


def kernel(x, y, device=0, _want_profile=False):
    from concourse.bass_utils import run_bass_kernel_spmd

    x = np.ascontiguousarray(np.asarray(x, dtype=np.float32))
    y = np.ascontiguousarray(np.asarray(y, dtype=np.float32))
    assert x.shape == (N, D) and y.shape == (N, D)

    bf = ml_dtypes.bfloat16
    xb = (-2.0 * x).astype(bf)                    # exact pow-2 scale
    yb = y.astype(bf)
    xsq = np.einsum("ij,ij->i", x, x, dtype=np.float32)
    ysq = np.einsum("ij,ij->i", y, y, dtype=np.float32)
    # ysqT[p, jt] = ysq[jt*128 + p]
    ysqT = np.ascontiguousarray(ysq.reshape(NJ, 128).T)

    nc = _get_nc()
    in_maps = [
        {
            "x": xb[c * SHARD:(c + 1) * SHARD],
            "y": yb[c * SHARD:(c + 1) * SHARD],
            "ysqT": ysqT,
        }
        for c in range(NCORES)
    ]
    try:
        res = run_bass_kernel_spmd(
            nc, in_maps, list(range(NCORES)), trace=_want_profile
        )
    except ModuleNotFoundError:
        res = run_bass_kernel_spmd(nc, in_maps, list(range(NCORES)))
    if _want_profile:
        _CACHE["exec_time_ns"] = getattr(res, "exec_time_ns", None)

    # per-core [1, SHARD] f32 row-mins
    parts = [
        np.asarray(res.results[c]["out"]).reshape(SHARD)
        for c in range(NCORES)
    ]
    m = np.concatenate(parts)  # [N] = min_j(||y_j||^2 - 2 x_i . y_j)

    md2 = xsq + m  # squared min distances (bf16-accurate)

    # exact fp32 top-K refinement: recompute candidate rows exactly so
    # bf16 rounding cannot flip the argmax.
    K = 128
    cand = np.argpartition(-md2, K)[:K]
    g = x[cand] @ y.T  # [K, N] exact fp32 (BLAS)
    d2 = xsq[cand][:, None] + ysq[None, :] - 2.0 * g
    cmin = d2.min(axis=1)
    best = int(np.argmax(cmin))
    max_id = int(cand[best])
    max_val = np.sqrt(np.maximum(cmin[best], 0.0), dtype=np.float32)

    return np.float32(max_val), np.int32(max_id)
